# revision 30
# baseline (speedup 1.0000x reference)
"""Trainium2 Bass kernel for DeepEdgeConvolution (gnn_message_passing).

Math (reference):
    bei = edge_nodes[:, src] + edge_nodes[:, dst]          # [B, E]
    bei = bei / row_sum (0 if empty row)
    h = BN1(relu(x @ W0 + b0)); h = BN2(relu(h @ W1 + b1)); h = h @ W2 + b2
    out = bei @ h                                          # [B, K]

Restructured: fold BN1 into (W1, b1) and BN2 into (W2, b2):
    a1 = relu(x @ W0 + b0)             (BN1 stats over E -> s1, t1)
    W1' = diag(s1) W1 ; b1' = t1 @ W1 + b1
    a2 = relu(a1 @ W1' + b1')          (BN2 stats over E -> s2, t2)
    W2' = diag(s2) W2 ; b2' = t2 @ W2 + b2
    out = diag(inv) [ (bei_raw @ a2) @ W2' + rs_raw x b2' ]

Sharding: edges across 8 cores; two streaming passes over x^T per core
(pass A: BN1 stats via bn_stats; pass B: recompute a1, then a2, G).

bei is computed on the HOST (sharding the columns of batch_edge_idcs per the
sharding hint): beiT is streamed as a dense packed input [128, NSUB*33] where
each 33-col block is [bei | 1] for one 128-edge subtile (edges on partitions).
Pad edges get all-zero columns (including the ones entry), which kills every
pad correction. Row sums rs / inv are exact small host-side inputs.

G accumulation (the [B,E]x[E,K] spmm): per 128-edge subtile,
    psG += a2_sub^T @ [bei_aug | a2_sub]    -> [H, 33 + H]
giving G^T (cols 0:32), sum(a2) (col 32, via the ones column) and the a2 Gram
matrix whose diagonal is sum(a2^2) -- one PSUM chain yields everything BN2 and
the final matmul need.  One AllReduce of [H, 34] follows; the epilogue is a
couple of tiny matmuls.
"""

import numpy as np

import concourse.bacc as bacc
import concourse.bass as bass
import concourse.tile as tile
from concourse import mybir
from concourse.bass_utils import run_bass_kernel_spmd
from concourse.masks import make_identity

f32 = mybir.dt.float32
bf16 = mybir.dt.bfloat16
i32 = mybir.dt.int32

NCORES = 8
B, D, H, KDIM = 32, 64, 128, 128
EPS = 1e-5
TILE = 512           # edges per tile
SUB = 128            # edges per matmul subtile
GATHER_BATCH = 2048  # edges per DMA chunk (ESH must be a multiple)

# compute dtype: "f32" (exact-ish) or "bf16" (fast).
COMPUTE_DT = "bf16"

BW = B + 1           # bei block width: [bei (32) | ones (1)]


def _np_dt(dt):
    if dt == bf16:
        import ml_dtypes
        return ml_dtypes.bfloat16
    return np.float32


def build_nc(ESH, N, E_total, dt_c=bf16, dt_en=None, debug=False):
    """Build the SPMD Bass program. ESH = padded edges per core."""
    del N, dt_en, debug
    assert ESH % GATHER_BATCH == 0
    NT = ESH // TILE          # tiles per core
    NSUB = ESH // SUB         # 128-edge subtiles per core
    NCH = ESH // GATHER_BATCH  # DMA chunks per core
    esh_real = E_total // NCORES
    assert E_total % NCORES == 0
    NS = TILE // SUB          # subtiles per tile (4)
    GW = BW + H               # gacc rhs width: [bei | 1 | a2]
    CCOL = GATHER_BATCH // 2  # packed x cols per chunk
    CT = GATHER_BATCH // TILE  # tiles per chunk (4)

    nc = bass.Bass()

    # ---- I/O ----
    xTi = nc.dram_tensor("xTi", [128, ESH // 2], dt_c, kind="ExternalInput")
    beiT = nc.dram_tensor("beiT", [128, NSUB * BW], dt_c, kind="ExternalInput")
    W0d = nc.dram_tensor("W0", [2 * D, H], dt_c, kind="ExternalInput")
    W1d = nc.dram_tensor("W1", [H, H], f32, kind="ExternalInput")
    W2d = nc.dram_tensor("W2", [H, KDIM], f32, kind="ExternalInput")
    b0cd = nc.dram_tensor("b0c", [H, 1], f32, kind="ExternalInput")
    b1rd = nc.dram_tensor("b1r", [1, H], f32, kind="ExternalInput")
    b2rd = nc.dram_tensor("b2r", [1, KDIM], f32, kind="ExternalInput")
    g0cd = nc.dram_tensor("g0c", [H, 1], f32, kind="ExternalInput")
    bt0cd = nc.dram_tensor("bt0c", [H, 1], f32, kind="ExternalInput")
    g1cd = nc.dram_tensor("g1c", [H, 1], f32, kind="ExternalInput")
    bt1cd = nc.dram_tensor("bt1c", [H, 1], f32, kind="ExternalInput")
    rsrd = nc.dram_tensor("rsr", [1, B], f32, kind="ExternalInput")
    invcd = nc.dram_tensor("invc", [B, 1], f32, kind="ExternalInput")
    qcd = nc.dram_tensor("qc", [H, 1], f32, kind="ExternalInput")
    outd = nc.dram_tensor("out", [B, KDIM], f32, kind="ExternalOutput")

    rg = [list(range(NCORES))]

    with tile.TileContext(nc) as tc:
        with (
            tc.tile_pool(name="const", bufs=1) as cpool,
            tc.tile_pool(name="xp", bufs=3) as xpool,
            tc.tile_pool(name="a1s", bufs=3) as a1spool,
            tc.tile_pool(name="a1l", bufs=4) as a1lpool,
            tc.tile_pool(name="a2p", bufs=3) as a2pool,
            tc.tile_pool(name="misc", bufs=2) as mpool,
            tc.tile_pool(name="psA", bufs=3, space="PSUM") as psA,
            tc.tile_pool(name="psB", bufs=2, space="PSUM") as psB,
            tc.tile_pool(name="psG", bufs=1, space="PSUM") as psG,
            tc.tile_pool(name="psS", bufs=2, space="PSUM") as psS,
            tc.tile_pool(name="dram", bufs=1, space="DRAM") as dpool,
        ):
            # ---- constants / params in SBUF ----
            w0sb = cpool.tile([128, H], dt_c)  # W0 duplicated on both halves
            nc.sync.dma_start(w0sb[:], W0d[:])
            w1sb = cpool.tile([H, H], f32)
            nc.sync.dma_start(w1sb[:], W1d[:])
            w2sb = cpool.tile([H, KDIM], f32)
            nc.sync.dma_start(w2sb[:], W2d[:])
            b0c = cpool.tile([H, 1], f32)
            nc.sync.dma_start(b0c[:], b0cd[:])
            b1r = cpool.tile([1, H], f32)
            nc.sync.dma_start(b1r[:], b1rd[:])
            b2r = cpool.tile([1, KDIM], f32)
            nc.sync.dma_start(b2r[:], b2rd[:])
            g0c = cpool.tile([H, 1], f32)
            nc.sync.dma_start(g0c[:], g0cd[:])
            bt0c = cpool.tile([H, 1], f32)
            nc.sync.dma_start(bt0c[:], bt0cd[:])
            g1c = cpool.tile([H, 1], f32)
            nc.sync.dma_start(g1c[:], g1cd[:])
            bt1c = cpool.tile([H, 1], f32)
            nc.sync.dma_start(bt1c[:], bt1cd[:])
            rsr = cpool.tile([1, B], f32)
            nc.sync.dma_start(rsr[:], rsrd[:])
            invc = cpool.tile([B, 1], f32)
            nc.sync.dma_start(invc[:], invcd[:])
            qc = cpool.tile([H, 1], f32)
            nc.sync.dma_start(qc[:], qcd[:])

            ones_row = cpool.tile([1, H], f32)
            nc.vector.memset(ones_row[:], 1.0)
            ones_c = cpool.tile([1, H], dt_c)
            nc.vector.memset(ones_c[:], 1.0)
            id128 = cpool.tile([128, 128], f32)
            make_identity(nc, id128[:])

            # mask for the one subtile that straddles the real/pad boundary
            pad_frac = esh_real % SUB
            edge_mask = None
            if pad_frac:
                pidx = cpool.tile([128, 1], i32)
                nc.gpsimd.iota(pidx[:], pattern=[[0, 1]], base=0,
                               channel_multiplier=1)
                pidx_f = cpool.tile([128, 1], f32)
                nc.vector.tensor_copy(pidx_f[:], pidx[:])
                edge_mask = cpool.tile([128, 1], f32)
                nc.vector.tensor_scalar(
                    edge_mask[:], pidx_f[:], float(pad_frac), None,
                    op0=mybir.AluOpType.is_lt)

            stats1 = cpool.tile([H, 6 * NT], f32)

            # bei resident in SBUF (DMA issues interleaved into pass A so the
            # SP issue queue doesn't delay the first x chunk)
            bei_sb = cpool.tile([128, NSUB * BW], dt_c)
            BCW = (GATHER_BATCH // SUB) * BW

            # a1 spill buffers in DRAM, one per chunk (precise DMA deps)
            a1d = [dpool.tile([128, GATHER_BATCH], dt_c, name=f"a1d{ch}")
                   for ch in range(NCH)]

            # ================= PASS A: BN1 stats; a1 spilled to DRAM ========
            for ch in range(NCH):
                xch = xpool.tile([128, CCOL], dt_c, tag="xch")
                nc.sync.dma_start(xch[:], xTi[:, ch * CCOL:(ch + 1) * CCOL])
                nc.sync.dma_start(
                    bei_sb[:, ch * BCW:(ch + 1) * BCW],
                    beiT[:, ch * BCW:(ch + 1) * BCW])
                a1ch = a1spool.tile([H, GATHER_BATCH], dt_c, tag="a1ch")
                for tp in range(GATHER_BATCH // (2 * TILE)):
                    for u in range(2):
                        t = ch * CT + 2 * tp + u
                        i = 2 * tp + u
                        z1 = psA.tile([H, TILE], f32, space="PSUM", tag="z1")
                        nc.tensor.matmul(
                            z1[:], lhsT=w0sb[u * D:(u + 1) * D, :],
                            rhs=xch[u * D:(u + 1) * D, tp * TILE:(tp + 1) * TILE],
                            start=True, stop=True)
                        a1 = a1ch[:, i * TILE:(i + 1) * TILE]
                        nc.scalar.activation(
                            a1, z1[:], mybir.ActivationFunctionType.Relu,
                            bias=b0c[:, 0:1])
                        pad_lo = esh_real - t * TILE
                        if pad_lo < TILE:
                            nc.vector.memset(
                                a1ch[:, i * TILE + max(pad_lo, 0):
                                     (i + 1) * TILE], 0.0)
                        nc.vector.bn_stats(stats1[:, 6 * t:6 * t + 6], a1)
                nc.sync.dma_start(a1d[ch][:], a1ch[:])

            # ---- AllReduce #1: BN1 sums (issued before the prelude so the
            # collective overlaps with pass-B z1/a1 work) ----
            mv1 = mpool.tile([H, 2], f32, tag="mv")
            nc.vector.bn_aggr(mv1[:], stats1[:])
            # raw sums over this shard (pads are zero -> exact)
            ar1 = mpool.tile([H, 2], f32, tag="ar")
            nc.scalar.mul(ar1[:, 0:1], mv1[:, 0:1], float(ESH))
            msq1 = mpool.tile([H, 1], f32, tag="msq")
            nc.vector.tensor_mul(msq1[:], mv1[:, 0:1], mv1[:, 0:1])
            nc.vector.tensor_add(msq1[:], msq1[:], mv1[:, 1:2])
            nc.scalar.mul(ar1[:, 1:2], msq1[:], float(ESH))

            cc1_in = dpool.tile([H, 2], f32)
            cc1_out = dpool.tile([H, 2], f32)
            nc.sync.dma_start(cc1_in[:], ar1[:])
            nc.gpsimd.collective_compute(
                "AllReduce", mybir.AluOpType.add, replica_groups=rg,
                ins=[cc1_in.opt()], outs=[cc1_out.opt()])
            gs1 = mpool.tile([H, 2], f32, tag="gs")
            nc.sync.dma_start(gs1[:], cc1_out[:])

            # mu, var, s1, t1
            mu1 = mpool.tile([H, 1], f32, tag="mu")
            nc.scalar.mul(mu1[:], gs1[:, 0:1], 1.0 / E_total)
            ex2 = mpool.tile([H, 1], f32, tag="ex2")
            nc.scalar.mul(ex2[:], gs1[:, 1:2], 1.0 / E_total)
            var1 = mpool.tile([H, 1], f32, tag="var")
            nc.vector.tensor_mul(var1[:], mu1[:], mu1[:])
            nc.vector.tensor_sub(var1[:], ex2[:], var1[:])
            sd1 = mpool.tile([H, 1], f32, tag="sd")
            nc.vector.tensor_scalar_add(sd1[:], var1[:], EPS)
            nc.scalar.sqrt(sd1[:], sd1[:])
            isd1 = mpool.tile([H, 1], f32, tag="isd")
            nc.vector.reciprocal(isd1[:], sd1[:])
            s1 = mpool.tile([H, 1], f32, tag="s1")
            nc.vector.tensor_mul(s1[:], g0c[:], isd1[:])
            t1 = mpool.tile([H, 1], f32, tag="t1")
            nc.vector.tensor_mul(t1[:], mu1[:], s1[:])
            nc.vector.tensor_sub(t1[:], bt0c[:], t1[:])

            # W1' (compute dtype); bias b1' enters z2 through the a1 shift
            # delta = diag(1/s1)(t1 + W1^-T b1), since
            # (relu(z1+b0) + delta)^T W1' = a1^T W1' + b1'  and
            # relu(z1+b0) + delta = max(z1 + (b0+delta), delta).
            w1p = cpool.tile([H, H], dt_c)
            nc.vector.tensor_scalar_mul(w1p[:], w1sb[:], s1[:, 0:1])
            is1 = mpool.tile([H, 1], f32, tag="is1")
            nc.vector.reciprocal(is1[:], s1[:])
            delta = cpool.tile([H, 1], f32)
            nc.vector.tensor_add(delta[:], t1[:], qc[:])
            nc.vector.tensor_mul(delta[:], delta[:], is1[:])

            # ============ PASS B: a2, G^T / sum2 / Gram accumulation ============
            gacc = psG.tile([H, GW], f32, space="PSUM", tag="gacc")
            for ch in range(NCH):
                a1ld = a1lpool.tile([H, GATHER_BATCH], dt_c, tag="a1ld")
                nc.sync.dma_start(a1ld[:], a1d[ch][:])
                for i in range(CT):
                    t = ch * CT + i
                    a1 = a1ld[:, i * TILE:(i + 1) * TILE]
                    nc.vector.tensor_scalar_add(a1, a1, delta[:, 0:1])
                    z2 = psB.tile([H, TILE], f32, space="PSUM", tag="z2")
                    for s in range(NS):
                        nc.tensor.matmul(
                            z2[:, s * H:(s + 1) * H],
                            lhsT=a1[:, s * SUB:(s + 1) * SUB],
                            rhs=w1p[:], start=True, stop=True)
                    # a2t: per subtile [bei (32) | ones (1) | a2 (128)]
                    a2t = a2pool.tile([128, NS * GW], dt_c, tag="a2t")
                    a2t3 = a2t[:].rearrange("p (g c) -> p g c", c=GW)
                    nc.vector.tensor_copy(
                        a2t3[:, :, 0:BW],
                        bei_sb[:, (t * NS) * BW:(t * NS + NS) * BW]
                        .rearrange("p (g c) -> p g c", c=BW))
                    nc.scalar.activation(
                        a2t3[:, :, BW:GW],
                        z2[:].rearrange("p (g c) -> p g c", c=H),
                        mybir.ActivationFunctionType.Relu)
                    # zero a2 for pad edges (bei cols are host-zeroed)
                    for s in range(NS):
                        pl = esh_real - (t * NS + s) * SUB
                        if pl <= 0:
                            nc.vector.memset(a2t3[:, s, BW:GW], 0.0)
                        elif pl < SUB:
                            nc.vector.tensor_scalar_mul(
                                a2t3[:, s, BW:GW], a2t3[:, s, BW:GW],
                                edge_mask[:, 0:1])
                    first = (t == 0)
                    last = (t == NT - 1)
                    for s in range(NS):
                        nc.tensor.matmul(
                            gacc[:],
                            lhsT=a2t[:, s * GW + BW:(s + 1) * GW],
                            rhs=a2t[:, s * GW:(s + 1) * GW],
                            start=(first and s == 0),
                            stop=(last and s == NS - 1),
                            skip_group_check=True)

            # ---- AllReduce #2: [G^T | sum2 | sumsq2] ----
            garr = mpool.tile([H, BW + 1], f32, tag="garr")
            nc.vector.tensor_copy(garr[:, 0:BW], gacc[:, 0:BW])
            scr = mpool.tile([128, 128], f32, tag="scr")
            nc.vector.tensor_mul(scr[:], gacc[:, BW:GW], id128[:])
            nc.vector.tensor_reduce(
                garr[:, BW:BW + 1], scr[:], mybir.AxisListType.X,
                mybir.AluOpType.add)

            cc2_in = dpool.tile([H, BW + 1], f32)
            cc2_out = dpool.tile([H, BW + 1], f32)
            nc.sync.dma_start(cc2_in[:], garr[:])
            nc.gpsimd.collective_compute(
                "AllReduce", mybir.AluOpType.add, replica_groups=rg,
                ins=[cc2_in.opt()], outs=[cc2_out.opt()])
            gall = mpool.tile([H, BW + 1], f32, tag="gall")
            nc.sync.dma_start(gall[:], cc2_out[:])

            # ---- epilogue ----
            mu2 = mpool.tile([H, 1], f32, tag="mu")
            nc.scalar.mul(mu2[:], gall[:, B:B + 1], 1.0 / E_total)
            ex2b = mpool.tile([H, 1], f32, tag="ex2")
            nc.scalar.mul(ex2b[:], gall[:, BW:BW + 1], 1.0 / E_total)
            var2 = mpool.tile([H, 1], f32, tag="var")
            nc.vector.tensor_mul(var2[:], mu2[:], mu2[:])
            nc.vector.tensor_sub(var2[:], ex2b[:], var2[:])
            sd2 = mpool.tile([H, 1], f32, tag="sd")
            nc.vector.tensor_scalar_add(sd2[:], var2[:], EPS)
            nc.scalar.sqrt(sd2[:], sd2[:])
            isd2 = mpool.tile([H, 1], f32, tag="isd")
            nc.vector.reciprocal(isd2[:], sd2[:])
            s2 = mpool.tile([H, 1], f32, tag="s1")
            nc.vector.tensor_mul(s2[:], g1c[:], isd2[:])
            t2 = mpool.tile([H, 1], f32, tag="t1")
            nc.vector.tensor_mul(t2[:], mu2[:], s2[:])
            nc.vector.tensor_sub(t2[:], bt1c[:], t2[:])

            w2p = mpool.tile([H, KDIM], f32, tag="w2p")
            nc.vector.tensor_scalar_mul(w2p[:], w2sb[:], s2[:, 0:1])
            pr2 = psS.tile([1, KDIM], f32, space="PSUM", tag="pss")
            nc.tensor.matmul(pr2[:], lhsT=t2[:], rhs=w2sb[:], start=True, stop=True)
            b2p_row = mpool.tile([1, KDIM], f32, tag="b2pr")
            nc.vector.tensor_add(b2p_row[:], pr2[:], b2r[:])

            out_ps = psS.tile([B, KDIM], f32, space="PSUM", tag="pss")
            nc.tensor.matmul(out_ps[:], lhsT=gall[:, 0:B], rhs=w2p[:],
                             start=True, stop=False)
            nc.tensor.matmul(out_ps[:], lhsT=rsr[:], rhs=b2p_row[:],
                             start=False, stop=True)
            outsb = mpool.tile([B, KDIM], f32, tag="outsb")
            nc.vector.tensor_scalar_mul(outsb[:], out_ps[:], invc[:, 0:1])
            nc.sync.dma_start(outd[:], outsb[:])

    # Legalize waits for walrus (TRN2: max 1 wait/instruction; extras are
    # spilled onto ldweights / event-semaphore instructions).
    import bass_rust as _br
    _br.move_matmul_waits_to_ldweights(nc.m)
    _br.generate_event_semaphores(nc)
    nc.finalize()
    return nc


def _ceil_to(x, m):
    return (x + m - 1) // m * m


def make_inputs(inputs, ESH, N, dt_c=bf16, dt_en=None):
    """Host-side shard/layout prep. Returns in_maps for run_bass_kernel_spmd."""
    del N, dt_en
    np_c = _np_dt(dt_c)
    en = np.asarray(inputs["edge_nodes"], dtype=np.float32)
    x = np.asarray(inputs["edge_feats"], dtype=np.float32)
    src = np.asarray(inputs["src"]).astype(np.int64)
    dst = np.asarray(inputs["dst"]).astype(np.int64)
    E = x.shape[0]
    Nn = en.shape[1]
    esh_real = E // NCORES
    NSUB = ESH // SUB

    # exact row sums via degree counts (en entries are 0/1)
    deg = (np.bincount(src, minlength=Nn) + np.bincount(dst, minlength=Nn))
    rs = en.astype(np.float64) @ deg.astype(np.float64)
    inv = np.where(rs > 0, 1.0 / np.where(rs > 0, rs, 1.0), 0.0)

    enT = en.T  # [N, B]

    common = dict(
        W0=np.vstack([np.asarray(inputs["W0"], np.float32)] * 2).astype(np_c),
        W1=np.asarray(inputs["W1"], np.float32),
        W2=np.asarray(inputs["W2"], np.float32),
        b0c=np.asarray(inputs["b0"], np.float32).reshape(H, 1),
        b1r=np.asarray(inputs["b1"], np.float32).reshape(1, H),
        b2r=np.asarray(inputs["b2"], np.float32).reshape(1, KDIM),
        g0c=np.asarray(inputs["g0"], np.float32).reshape(H, 1),
        bt0c=np.asarray(inputs["bt0"], np.float32).reshape(H, 1),
        g1c=np.asarray(inputs["g1"], np.float32).reshape(H, 1),
        bt1c=np.asarray(inputs["bt1"], np.float32).reshape(H, 1),
        rsr=rs.astype(np.float32).reshape(1, B),
        invc=inv.astype(np.float32).reshape(B, 1),
        qc=np.linalg.solve(
            np.asarray(inputs["W1"], np.float64).T,
            np.asarray(inputs["b1"], np.float64),
        ).astype(np.float32).reshape(H, 1),
    )

    in_maps = []
    for c in range(NCORES):
        lo = c * esh_real
        xs = x[lo:lo + esh_real]
        xT = np.zeros((D, ESH), np.float32)
        xT[:, :esh_real] = xs.T
        NTP = ESH // (2 * TILE)
        xTi = np.ascontiguousarray(
            xT.reshape(D, NTP, 2, TILE).transpose(2, 0, 1, 3).reshape(128, ESH // 2)
        ).astype(np_c)

        bei_aug = np.zeros((ESH, BW), np.float32)
        bei_aug[:esh_real, 0:B] = enT[src[lo:lo + esh_real]] + enT[dst[lo:lo + esh_real]]
        bei_aug[:esh_real, B] = 1.0
        beiT = np.ascontiguousarray(
            bei_aug.reshape(NSUB, 128, BW).transpose(1, 0, 2).reshape(128, NSUB * BW)
        ).astype(np_c)

        in_maps.append(dict(common, xTi=xTi, beiT=beiT))
    return in_maps


_NC_CACHE = {}


def kernel(**inputs):
    dt_c = bf16 if COMPUTE_DT == "bf16" else f32
    x = np.asarray(inputs["edge_feats"])
    en = np.asarray(inputs["edge_nodes"])
    E = x.shape[0]
    N = en.shape[1]
    ESH = _ceil_to(E // NCORES, GATHER_BATCH)
    key = (ESH, N, E, COMPUTE_DT)
    if key not in _NC_CACHE:
        _NC_CACHE[key] = build_nc(ESH, N, E, dt_c=dt_c)
    nc = _NC_CACHE[key]
    in_maps = make_inputs(inputs, ESH, N, dt_c=dt_c)
    res = run_bass_kernel_spmd(nc, in_maps, list(range(NCORES)))
    return np.asarray(res.results[0]["out"], np.float32)


# revision 35
# speedup vs baseline: 1.0947x; 1.0947x over previous
"""Trainium2 Bass kernel for DeepEdgeConvolution (gnn_message_passing).

Math (reference):
    bei = edge_nodes[:, src] + edge_nodes[:, dst]          # [B, E]
    bei = bei / row_sum (0 if empty row)
    h = BN1(relu(x @ W0 + b0)); h = BN2(relu(h @ W1 + b1)); h = h @ W2 + b2
    out = bei @ h                                          # [B, K]

Restructured: fold BN1 into (W1, b1) and BN2 into (W2, b2):
    a1 = relu(x @ W0 + b0)             (BN1 stats over E -> s1, t1)
    W1' = diag(s1) W1 ; b1' = t1 @ W1 + b1
    a2 = relu(a1 @ W1' + b1')          (BN2 stats over E -> s2, t2)
    W2' = diag(s2) W2 ; b2' = t2 @ W2 + b2
    out = diag(inv) [ (bei_raw @ a2) @ W2' + rs_raw x b2' ]

Sharding: edges across 8 cores; two streaming passes over x^T per core
(pass A: BN1 stats via bn_stats; pass B: recompute a1, then a2, G).

bei is computed on the HOST (sharding the columns of batch_edge_idcs per the
sharding hint): beiT is streamed as a dense packed input [128, NSUB*33] where
each 33-col block is [bei | 1] for one 128-edge subtile (edges on partitions).
Pad edges get all-zero columns (including the ones entry), which kills every
pad correction. Row sums rs / inv are exact small host-side inputs.

G accumulation (the [B,E]x[E,K] spmm): per 128-edge subtile,
    psG += a2_sub^T @ [bei_aug | a2_sub]    -> [H, 33 + H]
giving G^T (cols 0:32), sum(a2) (col 32, via the ones column) and the a2 Gram
matrix whose diagonal is sum(a2^2) -- one PSUM chain yields everything BN2 and
the final matmul need.  One AllReduce of [H, 34] follows; the epilogue is a
couple of tiny matmuls.
"""

import numpy as np

import concourse.bacc as bacc
import concourse.bass as bass
import concourse.tile as tile
from concourse import mybir
from concourse.bass_utils import run_bass_kernel_spmd
from concourse.masks import make_identity

f32 = mybir.dt.float32
bf16 = mybir.dt.bfloat16
i32 = mybir.dt.int32

NCORES = 8
B, D, H, KDIM = 32, 64, 128, 128
EPS = 1e-5
TILE = 512           # edges per tile
SUB = 128            # edges per matmul subtile
GATHER_BATCH = 2048  # edges per DMA chunk (ESH must be a multiple)

# compute dtype: "f32" (exact-ish) or "bf16" (fast).
COMPUTE_DT = "bf16"

BW = B + 1           # bei block width: [bei (32) | ones (1)]


def _np_dt(dt):
    if dt == bf16:
        import ml_dtypes
        return ml_dtypes.bfloat16
    return np.float32


def build_nc(ESH, N, E_total, dt_c=bf16, dt_en=None, debug=False):
    """Build the SPMD Bass program. ESH = padded edges per core."""
    del N, dt_en, debug
    assert ESH % GATHER_BATCH == 0
    NT = ESH // TILE          # tiles per core
    NSUB = ESH // SUB         # 128-edge subtiles per core
    NCH = ESH // GATHER_BATCH  # DMA chunks per core
    esh_real = E_total // NCORES
    assert E_total % NCORES == 0
    NS = TILE // SUB          # subtiles per tile (4)
    GW = BW + H               # gacc rhs width: [bei | 1 | a2]
    CCOL = GATHER_BATCH // 2  # packed x cols per chunk
    CT = GATHER_BATCH // TILE  # tiles per chunk (4)
    # chunks whose z1/a1 is recomputed inline between pass A and the BN1 fold
    # (AllReduce-independent PE work that hides the collective latency); these
    # chunks skip the DRAM a1 spill.
    PCH = min(8, NCH - 1) if NCH > 1 else 0

    nc = bass.Bass()

    # ---- I/O ----
    xTi = nc.dram_tensor("xTi", [128, ESH // 2], dt_c, kind="ExternalInput")
    beiT = nc.dram_tensor("beiT", [128, NSUB * BW], dt_c, kind="ExternalInput")
    W0d = nc.dram_tensor("W0", [2 * D, H], dt_c, kind="ExternalInput")
    W1d = nc.dram_tensor("W1", [H, H], f32, kind="ExternalInput")
    W2d = nc.dram_tensor("W2", [H, KDIM], f32, kind="ExternalInput")
    b0cd = nc.dram_tensor("b0c", [H, 1], f32, kind="ExternalInput")
    b1rd = nc.dram_tensor("b1r", [1, H], f32, kind="ExternalInput")
    b2rd = nc.dram_tensor("b2r", [1, KDIM], f32, kind="ExternalInput")
    g0cd = nc.dram_tensor("g0c", [H, 1], f32, kind="ExternalInput")
    bt0cd = nc.dram_tensor("bt0c", [H, 1], f32, kind="ExternalInput")
    g1cd = nc.dram_tensor("g1c", [H, 1], f32, kind="ExternalInput")
    bt1cd = nc.dram_tensor("bt1c", [H, 1], f32, kind="ExternalInput")
    rsrd = nc.dram_tensor("rsr", [1, B], f32, kind="ExternalInput")
    invcd = nc.dram_tensor("invc", [B, 1], f32, kind="ExternalInput")
    qcd = nc.dram_tensor("qc", [H, 1], f32, kind="ExternalInput")
    outd = nc.dram_tensor("out", [B, KDIM], f32, kind="ExternalOutput")

    rg = [list(range(NCORES))]

    with tile.TileContext(nc) as tc:
        with (
            tc.tile_pool(name="const", bufs=1) as cpool,
            tc.tile_pool(name="xp", bufs=3) as xpool,
            tc.tile_pool(name="a1s", bufs=3) as a1spool,
            tc.tile_pool(name="a1l", bufs=6) as a1lpool,
            tc.tile_pool(name="a1k", bufs=8) as a1kpool,
            tc.tile_pool(name="a2p", bufs=3) as a2pool,
            tc.tile_pool(name="misc", bufs=2) as mpool,
            tc.tile_pool(name="psA", bufs=3, space="PSUM") as psA,
            tc.tile_pool(name="psB", bufs=2, space="PSUM") as psB,
            tc.tile_pool(name="psG", bufs=1, space="PSUM") as psG,
            tc.tile_pool(name="psS", bufs=2, space="PSUM") as psS,
            tc.tile_pool(name="dram", bufs=1, space="DRAM") as dpool,
        ):
            # ---- constants / params in SBUF ----
            w0sb = cpool.tile([128, H], dt_c)  # W0 duplicated on both halves
            nc.sync.dma_start(w0sb[:], W0d[:])
            w1sb = cpool.tile([H, H], f32)
            nc.sync.dma_start(w1sb[:], W1d[:])
            w2sb = cpool.tile([H, KDIM], f32)
            nc.sync.dma_start(w2sb[:], W2d[:])
            b0c = cpool.tile([H, 1], f32)
            nc.sync.dma_start(b0c[:], b0cd[:])
            b1r = cpool.tile([1, H], f32)
            nc.sync.dma_start(b1r[:], b1rd[:])
            b2r = cpool.tile([1, KDIM], f32)
            nc.sync.dma_start(b2r[:], b2rd[:])
            g0c = cpool.tile([H, 1], f32)
            nc.sync.dma_start(g0c[:], g0cd[:])
            bt0c = cpool.tile([H, 1], f32)
            nc.sync.dma_start(bt0c[:], bt0cd[:])
            g1c = cpool.tile([H, 1], f32)
            nc.sync.dma_start(g1c[:], g1cd[:])
            bt1c = cpool.tile([H, 1], f32)
            nc.sync.dma_start(bt1c[:], bt1cd[:])
            rsr = cpool.tile([1, B], f32)
            nc.sync.dma_start(rsr[:], rsrd[:])
            invc = cpool.tile([B, 1], f32)
            nc.sync.dma_start(invc[:], invcd[:])
            qc = cpool.tile([H, 1], f32)
            nc.sync.dma_start(qc[:], qcd[:])

            ones_row = cpool.tile([1, H], f32)
            nc.vector.memset(ones_row[:], 1.0)
            ones_c = cpool.tile([1, H], dt_c)
            nc.vector.memset(ones_c[:], 1.0)
            id128 = cpool.tile([128, 128], f32)
            make_identity(nc, id128[:])

            # mask for the one subtile that straddles the real/pad boundary
            pad_frac = esh_real % SUB
            edge_mask = None
            if pad_frac:
                pidx = cpool.tile([128, 1], i32)
                nc.gpsimd.iota(pidx[:], pattern=[[0, 1]], base=0,
                               channel_multiplier=1)
                pidx_f = cpool.tile([128, 1], f32)
                nc.vector.tensor_copy(pidx_f[:], pidx[:])
                edge_mask = cpool.tile([128, 1], f32)
                nc.vector.tensor_scalar(
                    edge_mask[:], pidx_f[:], float(pad_frac), None,
                    op0=mybir.AluOpType.is_lt)

            stats1 = cpool.tile([H, 6 * NT], f32)

            # bei resident in SBUF (DMA issues interleaved into pass A so the
            # SP issue queue doesn't delay the first x chunk)
            bei_sb = cpool.tile([128, NSUB * BW], dt_c)
            BCW = (GATHER_BATCH // SUB) * BW

            # a1 spill buffers in DRAM, one per chunk (precise DMA deps)
            a1d = [dpool.tile([128, GATHER_BATCH], dt_c, name=f"a1d{ch}")
                   for ch in range(NCH)]

            # ================= PASS A: BN1 stats; a1 spilled to DRAM ========
            for ch in range(NCH):
                xch = xpool.tile([128, CCOL], dt_c, tag="xch")
                nc.sync.dma_start(xch[:], xTi[:, ch * CCOL:(ch + 1) * CCOL])
                nc.sync.dma_start(
                    bei_sb[:, ch * BCW:(ch + 1) * BCW],
                    beiT[:, ch * BCW:(ch + 1) * BCW])
                a1ch = a1spool.tile([H, GATHER_BATCH], dt_c, tag="a1ch")
                for tp in range(GATHER_BATCH // (2 * TILE)):
                    for u in range(2):
                        t = ch * CT + 2 * tp + u
                        i = 2 * tp + u
                        z1 = psA.tile([H, TILE], f32, space="PSUM", tag="z1")
                        nc.tensor.matmul(
                            z1[:], lhsT=w0sb[u * D:(u + 1) * D, :],
                            rhs=xch[u * D:(u + 1) * D, tp * TILE:(tp + 1) * TILE],
                            start=True, stop=True)
                        a1 = a1ch[:, i * TILE:(i + 1) * TILE]
                        nc.scalar.activation(
                            a1, z1[:], mybir.ActivationFunctionType.Relu,
                            bias=b0c[:, 0:1])
                        pad_lo = esh_real - t * TILE
                        if pad_lo < TILE:
                            nc.vector.memset(
                                a1ch[:, i * TILE + max(pad_lo, 0):
                                     (i + 1) * TILE], 0.0)
                        nc.vector.bn_stats(stats1[:, 6 * t:6 * t + 6], a1)
                if ch >= PCH:
                    nc.sync.dma_start(a1d[ch][:], a1ch[:])

            # ---- AllReduce #1: BN1 sums (issued before the prelude so the
            # collective overlaps with pass-B z1/a1 work) ----
            mv1 = mpool.tile([H, 2], f32, tag="mv")
            nc.vector.bn_aggr(mv1[:], stats1[:])
            # raw sums over this shard (pads are zero -> exact)
            ar1 = mpool.tile([H, 2], f32, tag="ar")
            nc.scalar.mul(ar1[:, 0:1], mv1[:, 0:1], float(ESH))
            msq1 = mpool.tile([H, 1], f32, tag="msq")
            nc.vector.tensor_mul(msq1[:], mv1[:, 0:1], mv1[:, 0:1])
            nc.vector.tensor_add(msq1[:], msq1[:], mv1[:, 1:2])
            nc.scalar.mul(ar1[:, 1:2], msq1[:], float(ESH))

            cc1_in = dpool.tile([H, 2], f32)
            cc1_out = dpool.tile([H, 2], f32)
            nc.sync.dma_start(cc1_in[:], ar1[:])
            nc.gpsimd.collective_compute(
                "AllReduce", mybir.AluOpType.add, replica_groups=rg,
                ins=[cc1_in.opt()], outs=[cc1_out.opt()])
            gs1 = mpool.tile([H, 2], f32, tag="gs")
            nc.sync.dma_start(gs1[:], cc1_out[:])

            # ---- prelude: recompute z1/a1 of the first PCH chunks while the
            # collective is in flight ----
            a1_keep = {}
            for ch in range(PCH):
                xch = xpool.tile([128, CCOL], dt_c, tag="xch")
                nc.sync.dma_start(xch[:], xTi[:, ch * CCOL:(ch + 1) * CCOL])
                a1k = a1kpool.tile([H, GATHER_BATCH], dt_c, tag="a1k")
                for tp in range(GATHER_BATCH // (2 * TILE)):
                    for u in range(2):
                        i = 2 * tp + u
                        z1 = psA.tile([H, TILE], f32, space="PSUM", tag="z1")
                        nc.tensor.matmul(
                            z1[:], lhsT=w0sb[u * D:(u + 1) * D, :],
                            rhs=xch[u * D:(u + 1) * D, tp * TILE:(tp + 1) * TILE],
                            start=True, stop=True)
                        nc.vector.tensor_scalar(
                            a1k[:, i * TILE:(i + 1) * TILE], z1[:],
                            b0c[:, 0:1], 0.0,
                            op0=mybir.AluOpType.add, op1=mybir.AluOpType.max)
                a1_keep[ch] = a1k

            # mu, var, s1, t1
            mu1 = mpool.tile([H, 1], f32, tag="mu")
            nc.scalar.mul(mu1[:], gs1[:, 0:1], 1.0 / E_total)
            ex2 = mpool.tile([H, 1], f32, tag="ex2")
            nc.scalar.mul(ex2[:], gs1[:, 1:2], 1.0 / E_total)
            var1 = mpool.tile([H, 1], f32, tag="var")
            nc.vector.tensor_mul(var1[:], mu1[:], mu1[:])
            nc.vector.tensor_sub(var1[:], ex2[:], var1[:])
            sd1 = mpool.tile([H, 1], f32, tag="sd")
            nc.vector.tensor_scalar_add(sd1[:], var1[:], EPS)
            nc.scalar.sqrt(sd1[:], sd1[:])
            isd1 = mpool.tile([H, 1], f32, tag="isd")
            nc.vector.reciprocal(isd1[:], sd1[:])
            s1 = mpool.tile([H, 1], f32, tag="s1")
            nc.vector.tensor_mul(s1[:], g0c[:], isd1[:])
            t1 = mpool.tile([H, 1], f32, tag="t1")
            nc.vector.tensor_mul(t1[:], mu1[:], s1[:])
            nc.vector.tensor_sub(t1[:], bt0c[:], t1[:])

            # W1' (compute dtype); bias b1' enters z2 through the a1 shift
            # delta = diag(1/s1)(t1 + W1^-T b1), since
            # (relu(z1+b0) + delta)^T W1' = a1^T W1' + b1'  and
            # relu(z1+b0) + delta = max(z1 + (b0+delta), delta).
            w1p = cpool.tile([H, H], dt_c)
            nc.vector.tensor_scalar_mul(w1p[:], w1sb[:], s1[:, 0:1])
            is1 = mpool.tile([H, 1], f32, tag="is1")
            nc.vector.reciprocal(is1[:], s1[:])
            delta = cpool.tile([H, 1], f32)
            nc.vector.tensor_add(delta[:], t1[:], qc[:])
            nc.vector.tensor_mul(delta[:], delta[:], is1[:])

            # ============ PASS B: a2, G^T / sum2 / Gram accumulation ============
            gacc = psG.tile([H, GW], f32, space="PSUM", tag="gacc")
            for ch in range(NCH):
                if ch in a1_keep:
                    a1ld = a1_keep.pop(ch)
                else:
                    a1ld = a1lpool.tile([H, GATHER_BATCH], dt_c, tag="a1ld")
                    nc.sync.dma_start(a1ld[:], a1d[ch][:])
                for i in range(CT):
                    t = ch * CT + i
                    a1 = a1ld[:, i * TILE:(i + 1) * TILE]
                    nc.vector.tensor_scalar_add(a1, a1, delta[:, 0:1])
                    z2 = psB.tile([H, TILE], f32, space="PSUM", tag="z2")
                    for s in range(NS):
                        nc.tensor.matmul(
                            z2[:, s * H:(s + 1) * H],
                            lhsT=a1[:, s * SUB:(s + 1) * SUB],
                            rhs=w1p[:], start=True, stop=True)
                    # a2t: per subtile [bei (32) | ones (1) | a2 (128)]
                    a2t = a2pool.tile([128, NS * GW], dt_c, tag="a2t")
                    a2t3 = a2t[:].rearrange("p (g c) -> p g c", c=GW)
                    nc.vector.tensor_copy(
                        a2t3[:, :, 0:BW],
                        bei_sb[:, (t * NS) * BW:(t * NS + NS) * BW]
                        .rearrange("p (g c) -> p g c", c=BW))
                    nc.scalar.activation(
                        a2t3[:, :, BW:GW],
                        z2[:].rearrange("p (g c) -> p g c", c=H),
                        mybir.ActivationFunctionType.Relu)
                    # zero a2 for pad edges (bei cols are host-zeroed)
                    for s in range(NS):
                        pl = esh_real - (t * NS + s) * SUB
                        if pl <= 0:
                            nc.vector.memset(a2t3[:, s, BW:GW], 0.0)
                        elif pl < SUB:
                            nc.vector.tensor_scalar_mul(
                                a2t3[:, s, BW:GW], a2t3[:, s, BW:GW],
                                edge_mask[:, 0:1])
                    first = (t == 0)
                    last = (t == NT - 1)
                    for s in range(NS):
                        nc.tensor.matmul(
                            gacc[:],
                            lhsT=a2t[:, s * GW + BW:(s + 1) * GW],
                            rhs=a2t[:, s * GW:(s + 1) * GW],
                            start=(first and s == 0),
                            stop=(last and s == NS - 1),
                            skip_group_check=True)

            # ---- AllReduce #2: [G^T | sum2 | sumsq2] ----
            garr = mpool.tile([H, BW + 1], f32, tag="garr")
            nc.vector.tensor_copy(garr[:, 0:BW], gacc[:, 0:BW])
            scr = mpool.tile([128, 128], f32, tag="scr")
            nc.vector.tensor_mul(scr[:], gacc[:, BW:GW], id128[:])
            nc.vector.tensor_reduce(
                garr[:, BW:BW + 1], scr[:], mybir.AxisListType.X,
                mybir.AluOpType.add)

            cc2_in = dpool.tile([H, BW + 1], f32)
            cc2_out = dpool.tile([H, BW + 1], f32)
            nc.sync.dma_start(cc2_in[:], garr[:])
            nc.gpsimd.collective_compute(
                "AllReduce", mybir.AluOpType.add, replica_groups=rg,
                ins=[cc2_in.opt()], outs=[cc2_out.opt()])
            gall = mpool.tile([H, BW + 1], f32, tag="gall")
            nc.sync.dma_start(gall[:], cc2_out[:])

            # ---- epilogue ----
            mu2 = mpool.tile([H, 1], f32, tag="mu")
            nc.scalar.mul(mu2[:], gall[:, B:B + 1], 1.0 / E_total)
            ex2b = mpool.tile([H, 1], f32, tag="ex2")
            nc.scalar.mul(ex2b[:], gall[:, BW:BW + 1], 1.0 / E_total)
            var2 = mpool.tile([H, 1], f32, tag="var")
            nc.vector.tensor_mul(var2[:], mu2[:], mu2[:])
            nc.vector.tensor_sub(var2[:], ex2b[:], var2[:])
            sd2 = mpool.tile([H, 1], f32, tag="sd")
            nc.vector.tensor_scalar_add(sd2[:], var2[:], EPS)
            nc.scalar.sqrt(sd2[:], sd2[:])
            isd2 = mpool.tile([H, 1], f32, tag="isd")
            nc.vector.reciprocal(isd2[:], sd2[:])
            s2 = mpool.tile([H, 1], f32, tag="s1")
            nc.vector.tensor_mul(s2[:], g1c[:], isd2[:])
            t2 = mpool.tile([H, 1], f32, tag="t1")
            nc.vector.tensor_mul(t2[:], mu2[:], s2[:])
            nc.vector.tensor_sub(t2[:], bt1c[:], t2[:])

            w2p = mpool.tile([H, KDIM], f32, tag="w2p")
            nc.vector.tensor_scalar_mul(w2p[:], w2sb[:], s2[:, 0:1])
            pr2 = psS.tile([1, KDIM], f32, space="PSUM", tag="pss")
            nc.tensor.matmul(pr2[:], lhsT=t2[:], rhs=w2sb[:], start=True, stop=True)
            b2p_row = mpool.tile([1, KDIM], f32, tag="b2pr")
            nc.vector.tensor_add(b2p_row[:], pr2[:], b2r[:])

            out_ps = psS.tile([B, KDIM], f32, space="PSUM", tag="pss")
            nc.tensor.matmul(out_ps[:], lhsT=gall[:, 0:B], rhs=w2p[:],
                             start=True, stop=False)
            nc.tensor.matmul(out_ps[:], lhsT=rsr[:], rhs=b2p_row[:],
                             start=False, stop=True)
            outsb = mpool.tile([B, KDIM], f32, tag="outsb")
            nc.vector.tensor_scalar_mul(outsb[:], out_ps[:], invc[:, 0:1])
            nc.sync.dma_start(outd[:], outsb[:])

    # Legalize waits for walrus (TRN2: max 1 wait/instruction; extras are
    # spilled onto ldweights / event-semaphore instructions).
    import bass_rust as _br
    _br.move_matmul_waits_to_ldweights(nc.m)
    _br.generate_event_semaphores(nc)
    nc.finalize()
    return nc


def _ceil_to(x, m):
    return (x + m - 1) // m * m


def make_inputs(inputs, ESH, N, dt_c=bf16, dt_en=None):
    """Host-side shard/layout prep. Returns in_maps for run_bass_kernel_spmd."""
    del N, dt_en
    np_c = _np_dt(dt_c)
    en = np.asarray(inputs["edge_nodes"], dtype=np.float32)
    x = np.asarray(inputs["edge_feats"], dtype=np.float32)
    src = np.asarray(inputs["src"]).astype(np.int64)
    dst = np.asarray(inputs["dst"]).astype(np.int64)
    E = x.shape[0]
    Nn = en.shape[1]
    esh_real = E // NCORES
    NSUB = ESH // SUB

    # exact row sums via degree counts (en entries are 0/1)
    deg = (np.bincount(src, minlength=Nn) + np.bincount(dst, minlength=Nn))
    rs = en.astype(np.float64) @ deg.astype(np.float64)
    inv = np.where(rs > 0, 1.0 / np.where(rs > 0, rs, 1.0), 0.0)

    enT = en.T  # [N, B]

    common = dict(
        W0=np.vstack([np.asarray(inputs["W0"], np.float32)] * 2).astype(np_c),
        W1=np.asarray(inputs["W1"], np.float32),
        W2=np.asarray(inputs["W2"], np.float32),
        b0c=np.asarray(inputs["b0"], np.float32).reshape(H, 1),
        b1r=np.asarray(inputs["b1"], np.float32).reshape(1, H),
        b2r=np.asarray(inputs["b2"], np.float32).reshape(1, KDIM),
        g0c=np.asarray(inputs["g0"], np.float32).reshape(H, 1),
        bt0c=np.asarray(inputs["bt0"], np.float32).reshape(H, 1),
        g1c=np.asarray(inputs["g1"], np.float32).reshape(H, 1),
        bt1c=np.asarray(inputs["bt1"], np.float32).reshape(H, 1),
        rsr=rs.astype(np.float32).reshape(1, B),
        invc=inv.astype(np.float32).reshape(B, 1),
        qc=np.linalg.solve(
            np.asarray(inputs["W1"], np.float64).T,
            np.asarray(inputs["b1"], np.float64),
        ).astype(np.float32).reshape(H, 1),
    )

    in_maps = []
    for c in range(NCORES):
        lo = c * esh_real
        xs = x[lo:lo + esh_real]
        xT = np.zeros((D, ESH), np.float32)
        xT[:, :esh_real] = xs.T
        NTP = ESH // (2 * TILE)
        xTi = np.ascontiguousarray(
            xT.reshape(D, NTP, 2, TILE).transpose(2, 0, 1, 3).reshape(128, ESH // 2)
        ).astype(np_c)

        bei_aug = np.zeros((ESH, BW), np.float32)
        bei_aug[:esh_real, 0:B] = enT[src[lo:lo + esh_real]] + enT[dst[lo:lo + esh_real]]
        bei_aug[:esh_real, B] = 1.0
        beiT = np.ascontiguousarray(
            bei_aug.reshape(NSUB, 128, BW).transpose(1, 0, 2).reshape(128, NSUB * BW)
        ).astype(np_c)

        in_maps.append(dict(common, xTi=xTi, beiT=beiT))
    return in_maps


_NC_CACHE = {}


def kernel(**inputs):
    dt_c = bf16 if COMPUTE_DT == "bf16" else f32
    x = np.asarray(inputs["edge_feats"])
    en = np.asarray(inputs["edge_nodes"])
    E = x.shape[0]
    N = en.shape[1]
    ESH = _ceil_to(E // NCORES, GATHER_BATCH)
    key = (ESH, N, E, COMPUTE_DT)
    if key not in _NC_CACHE:
        _NC_CACHE[key] = build_nc(ESH, N, E, dt_c=dt_c)
    nc = _NC_CACHE[key]
    in_maps = make_inputs(inputs, ESH, N, dt_c=dt_c)
    res = run_bass_kernel_spmd(nc, in_maps, list(range(NCORES)))
    return np.asarray(res.results[0]["out"], np.float32)


# revision 37
# speedup vs baseline: 1.0999x; 1.0048x over previous
"""Trainium2 Bass kernel for DeepEdgeConvolution (gnn_message_passing).

Math (reference):
    bei = edge_nodes[:, src] + edge_nodes[:, dst]          # [B, E]
    bei = bei / row_sum (0 if empty row)
    h = BN1(relu(x @ W0 + b0)); h = BN2(relu(h @ W1 + b1)); h = h @ W2 + b2
    out = bei @ h                                          # [B, K]

Restructured: fold BN1 into (W1, b1) and BN2 into (W2, b2):
    a1 = relu(x @ W0 + b0)             (BN1 stats over E -> s1, t1)
    W1' = diag(s1) W1 ; b1' = t1 @ W1 + b1
    a2 = relu(a1 @ W1' + b1')          (BN2 stats over E -> s2, t2)
    W2' = diag(s2) W2 ; b2' = t2 @ W2 + b2
    out = diag(inv) [ (bei_raw @ a2) @ W2' + rs_raw x b2' ]

Sharding: edges across 8 cores; two streaming passes over x^T per core
(pass A: BN1 stats via bn_stats; pass B: recompute a1, then a2, G).

bei is computed on the HOST (sharding the columns of batch_edge_idcs per the
sharding hint): beiT is streamed as a dense packed input [128, NSUB*33] where
each 33-col block is [bei | 1] for one 128-edge subtile (edges on partitions).
Pad edges get all-zero columns (including the ones entry), which kills every
pad correction. Row sums rs / inv are exact small host-side inputs.

G accumulation (the [B,E]x[E,K] spmm): per 128-edge subtile,
    psG += a2_sub^T @ [bei_aug | a2_sub]    -> [H, 33 + H]
giving G^T (cols 0:32), sum(a2) (col 32, via the ones column) and the a2 Gram
matrix whose diagonal is sum(a2^2) -- one PSUM chain yields everything BN2 and
the final matmul need.  One AllReduce of [H, 34] follows; the epilogue is a
couple of tiny matmuls.
"""

import numpy as np

import concourse.bacc as bacc
import concourse.bass as bass
import concourse.tile as tile
from concourse import mybir
from concourse.bass_utils import run_bass_kernel_spmd
from concourse.masks import make_identity

f32 = mybir.dt.float32
bf16 = mybir.dt.bfloat16
i32 = mybir.dt.int32

NCORES = 8
B, D, H, KDIM = 32, 64, 128, 128
EPS = 1e-5
TILE = 512           # edges per tile
SUB = 128            # edges per matmul subtile
GATHER_BATCH = 2048  # edges per DMA chunk (ESH must be a multiple)

# compute dtype: "f32" (exact-ish) or "bf16" (fast).
COMPUTE_DT = "bf16"

BW = B + 1           # bei block width: [bei (32) | ones (1)]


def _np_dt(dt):
    if dt == bf16:
        import ml_dtypes
        return ml_dtypes.bfloat16
    return np.float32


def build_nc(ESH, N, E_total, dt_c=bf16, dt_en=None, debug=False):
    """Build the SPMD Bass program. ESH = padded edges per core."""
    del N, dt_en, debug
    assert ESH % GATHER_BATCH == 0
    NT = ESH // TILE          # tiles per core
    NSUB = ESH // SUB         # 128-edge subtiles per core
    NCH = ESH // GATHER_BATCH  # DMA chunks per core
    esh_real = E_total // NCORES
    assert E_total % NCORES == 0
    NS = TILE // SUB          # subtiles per tile (4)
    GW = BW + H               # gacc rhs width: [bei | 1 | a2]
    CCOL = GATHER_BATCH // 2  # packed x cols per chunk
    CT = GATHER_BATCH // TILE  # tiles per chunk (4)
    # chunks whose z1/a1 is recomputed inline between pass A and the BN1 fold
    # (AllReduce-independent PE work that hides the collective latency); these
    # chunks skip the DRAM a1 spill.
    PCH = min(12, NCH - 1) if NCH > 1 else 0

    nc = bass.Bass()

    # ---- I/O ----
    xTi = nc.dram_tensor("xTi", [128, ESH // 2], dt_c, kind="ExternalInput")
    beiT = nc.dram_tensor("beiT", [128, NSUB * BW], dt_c, kind="ExternalInput")
    W0d = nc.dram_tensor("W0", [2 * D, H], dt_c, kind="ExternalInput")
    W1d = nc.dram_tensor("W1", [H, H], f32, kind="ExternalInput")
    W2d = nc.dram_tensor("W2", [H, KDIM], f32, kind="ExternalInput")
    b0cd = nc.dram_tensor("b0c", [H, 1], f32, kind="ExternalInput")
    b2rd = nc.dram_tensor("b2r", [1, KDIM], f32, kind="ExternalInput")
    g0cd = nc.dram_tensor("g0c", [H, 1], f32, kind="ExternalInput")
    bt0cd = nc.dram_tensor("bt0c", [H, 1], f32, kind="ExternalInput")
    g1cd = nc.dram_tensor("g1c", [H, 1], f32, kind="ExternalInput")
    bt1cd = nc.dram_tensor("bt1c", [H, 1], f32, kind="ExternalInput")
    rsrd = nc.dram_tensor("rsr", [1, B], f32, kind="ExternalInput")
    invcd = nc.dram_tensor("invc", [B, 1], f32, kind="ExternalInput")
    qcd = nc.dram_tensor("qc", [H, 1], f32, kind="ExternalInput")
    outd = nc.dram_tensor("out", [B, KDIM], f32, kind="ExternalOutput")

    rg = [list(range(NCORES))]

    with tile.TileContext(nc) as tc:
        with (
            tc.tile_pool(name="const", bufs=1) as cpool,
            tc.tile_pool(name="xp", bufs=3) as xpool,
            tc.tile_pool(name="a1s", bufs=3) as a1spool,
            tc.tile_pool(name="a1l", bufs=6) as a1lpool,
            tc.tile_pool(name="a1k", bufs=12) as a1kpool,
            tc.tile_pool(name="a2p", bufs=3) as a2pool,
            tc.tile_pool(name="misc", bufs=2) as mpool,
            tc.tile_pool(name="psA", bufs=4, space="PSUM") as psA,
            tc.tile_pool(name="psB", bufs=2, space="PSUM") as psB,
            tc.tile_pool(name="psG", bufs=1, space="PSUM") as psG,
            tc.tile_pool(name="psS", bufs=1, space="PSUM") as psS,
            tc.tile_pool(name="dram", bufs=1, space="DRAM") as dpool,
        ):
            # ---- constants / params in SBUF ----
            w0sb = cpool.tile([128, H], dt_c)  # W0 duplicated on both halves
            nc.sync.dma_start(w0sb[:], W0d[:])
            b0c = cpool.tile([H, 1], f32)
            nc.sync.dma_start(b0c[:], b0cd[:])

            stats1 = cpool.tile([H, 6 * NT], f32)

            # bei resident in SBUF (DMA issues interleaved into pass A so the
            # SP issue queue doesn't delay the first x chunk)
            bei_sb = cpool.tile([128, NSUB * BW], dt_c)
            BCW = (GATHER_BATCH // SUB) * BW

            # a1 spill buffers in DRAM, one per chunk (precise DMA deps)
            a1d = [dpool.tile([128, GATHER_BATCH], dt_c, name=f"a1d{ch}")
                   for ch in range(NCH)]

            # ================= PASS A: BN1 stats; a1 spilled to DRAM ========
            for ch in range(NCH):
                xch = xpool.tile([128, CCOL], dt_c, tag="xch")
                nc.sync.dma_start(xch[:], xTi[:, ch * CCOL:(ch + 1) * CCOL])
                nc.sync.dma_start(
                    bei_sb[:, ch * BCW:(ch + 1) * BCW],
                    beiT[:, ch * BCW:(ch + 1) * BCW])
                a1ch = a1spool.tile([H, GATHER_BATCH], dt_c, tag="a1ch")
                for tp in range(GATHER_BATCH // (2 * TILE)):
                    for u in range(2):
                        t = ch * CT + 2 * tp + u
                        i = 2 * tp + u
                        z1 = psA.tile([H, TILE], f32, space="PSUM", tag="z1")
                        nc.tensor.matmul(
                            z1[:], lhsT=w0sb[u * D:(u + 1) * D, :],
                            rhs=xch[u * D:(u + 1) * D, tp * TILE:(tp + 1) * TILE],
                            start=True, stop=True)
                        a1 = a1ch[:, i * TILE:(i + 1) * TILE]
                        nc.scalar.activation(
                            a1, z1[:], mybir.ActivationFunctionType.Relu,
                            bias=b0c[:, 0:1])
                        pad_lo = esh_real - t * TILE
                        if pad_lo < TILE:
                            nc.vector.memset(
                                a1ch[:, i * TILE + max(pad_lo, 0):
                                     (i + 1) * TILE], 0.0)
                        nc.vector.bn_stats(stats1[:, 6 * t:6 * t + 6], a1)
                if ch >= PCH:
                    nc.sync.dma_start(a1d[ch][:], a1ch[:])

            # ---- AllReduce #1: BN1 sums (issued before the prelude so the
            # collective overlaps with pass-B z1/a1 work) ----
            mv1 = mpool.tile([H, 2], f32, tag="mv")
            nc.vector.bn_aggr(mv1[:], stats1[:])
            # raw sums over this shard (pads are zero -> exact)
            ar1 = mpool.tile([H, 2], f32, tag="ar")
            nc.scalar.mul(ar1[:, 0:1], mv1[:, 0:1], float(ESH))
            msq1 = mpool.tile([H, 1], f32, tag="msq")
            nc.vector.tensor_mul(msq1[:], mv1[:, 0:1], mv1[:, 0:1])
            nc.vector.tensor_add(msq1[:], msq1[:], mv1[:, 1:2])
            nc.scalar.mul(ar1[:, 1:2], msq1[:], float(ESH))

            cc1_in = dpool.tile([H, 2], f32)
            cc1_out = dpool.tile([H, 2], f32)
            nc.sync.dma_start(cc1_in[:], ar1[:])
            nc.gpsimd.collective_compute(
                "AllReduce", mybir.AluOpType.add, replica_groups=rg,
                ins=[cc1_in.opt()], outs=[cc1_out.opt()])
            gs1 = mpool.tile([H, 2], f32, tag="gs")
            nc.sync.dma_start(gs1[:], cc1_out[:])

            # ---- prelude: recompute z1/a1 of the first PCH chunks while the
            # collective is in flight ----
            a1_keep = {}
            for ch in range(PCH):
                xch = xpool.tile([128, CCOL], dt_c, tag="xch")
                nc.sync.dma_start(xch[:], xTi[:, ch * CCOL:(ch + 1) * CCOL])
                a1k = a1kpool.tile([H, GATHER_BATCH], dt_c, tag="a1k")
                for tp in range(GATHER_BATCH // (2 * TILE)):
                    for u in range(2):
                        i = 2 * tp + u
                        z1 = psA.tile([H, TILE], f32, space="PSUM", tag="z1")
                        nc.tensor.matmul(
                            z1[:], lhsT=w0sb[u * D:(u + 1) * D, :],
                            rhs=xch[u * D:(u + 1) * D, tp * TILE:(tp + 1) * TILE],
                            start=True, stop=True)
                        nc.vector.tensor_scalar(
                            a1k[:, i * TILE:(i + 1) * TILE], z1[:],
                            b0c[:, 0:1], 0.0,
                            op0=mybir.AluOpType.add, op1=mybir.AluOpType.max)
                a1_keep[ch] = a1k

            # ---- deferred params (not needed until the fold / epilogue) ----
            w1sb = cpool.tile([H, H], f32)
            nc.sync.dma_start(w1sb[:], W1d[:])
            w2sb = cpool.tile([H, KDIM], f32)
            nc.sync.dma_start(w2sb[:], W2d[:])
            b2r = cpool.tile([1, KDIM], f32)
            nc.sync.dma_start(b2r[:], b2rd[:])
            g0c = cpool.tile([H, 1], f32)
            nc.sync.dma_start(g0c[:], g0cd[:])
            bt0c = cpool.tile([H, 1], f32)
            nc.sync.dma_start(bt0c[:], bt0cd[:])
            g1c = cpool.tile([H, 1], f32)
            nc.sync.dma_start(g1c[:], g1cd[:])
            bt1c = cpool.tile([H, 1], f32)
            nc.sync.dma_start(bt1c[:], bt1cd[:])
            rsr = cpool.tile([1, B], f32)
            nc.sync.dma_start(rsr[:], rsrd[:])
            invc = cpool.tile([B, 1], f32)
            nc.sync.dma_start(invc[:], invcd[:])
            qc = cpool.tile([H, 1], f32)
            nc.sync.dma_start(qc[:], qcd[:])
            id128 = cpool.tile([128, 128], f32)
            make_identity(nc, id128[:])
            pad_frac = esh_real % SUB
            edge_mask = None
            if pad_frac:
                pidx = cpool.tile([128, 1], i32)
                nc.gpsimd.iota(pidx[:], pattern=[[0, 1]], base=0,
                               channel_multiplier=1)
                pidx_f = cpool.tile([128, 1], f32)
                nc.vector.tensor_copy(pidx_f[:], pidx[:])
                edge_mask = cpool.tile([128, 1], f32)
                nc.vector.tensor_scalar(
                    edge_mask[:], pidx_f[:], float(pad_frac), None,
                    op0=mybir.AluOpType.is_lt)

            # mu, var, s1, t1
            mu1 = mpool.tile([H, 1], f32, tag="mu")
            nc.scalar.mul(mu1[:], gs1[:, 0:1], 1.0 / E_total)
            ex2 = mpool.tile([H, 1], f32, tag="ex2")
            nc.scalar.mul(ex2[:], gs1[:, 1:2], 1.0 / E_total)
            var1 = mpool.tile([H, 1], f32, tag="var")
            nc.vector.tensor_mul(var1[:], mu1[:], mu1[:])
            nc.vector.tensor_sub(var1[:], ex2[:], var1[:])
            sd1 = mpool.tile([H, 1], f32, tag="sd")
            nc.vector.tensor_scalar_add(sd1[:], var1[:], EPS)
            nc.scalar.sqrt(sd1[:], sd1[:])
            isd1 = mpool.tile([H, 1], f32, tag="isd")
            nc.vector.reciprocal(isd1[:], sd1[:])
            s1 = mpool.tile([H, 1], f32, tag="s1")
            nc.vector.tensor_mul(s1[:], g0c[:], isd1[:])
            t1 = mpool.tile([H, 1], f32, tag="t1")
            nc.vector.tensor_mul(t1[:], mu1[:], s1[:])
            nc.vector.tensor_sub(t1[:], bt0c[:], t1[:])

            # W1' (compute dtype); bias b1' enters z2 through the a1 shift
            # delta = diag(1/s1)(t1 + W1^-T b1), since
            # (relu(z1+b0) + delta)^T W1' = a1^T W1' + b1'  and
            # relu(z1+b0) + delta = max(z1 + (b0+delta), delta).
            w1p = cpool.tile([H, H], dt_c)
            nc.vector.tensor_scalar_mul(w1p[:], w1sb[:], s1[:, 0:1])
            is1 = mpool.tile([H, 1], f32, tag="is1")
            nc.vector.reciprocal(is1[:], s1[:])
            delta = cpool.tile([H, 1], f32)
            nc.vector.tensor_add(delta[:], t1[:], qc[:])
            nc.vector.tensor_mul(delta[:], delta[:], is1[:])

            # ============ PASS B: a2, G^T / sum2 / Gram accumulation ============
            gacc = psG.tile([H, GW], f32, space="PSUM", tag="gacc")
            for ch in range(NCH):
                if ch in a1_keep:
                    a1ld = a1_keep.pop(ch)
                else:
                    a1ld = a1lpool.tile([H, GATHER_BATCH], dt_c, tag="a1ld")
                    nc.sync.dma_start(a1ld[:], a1d[ch][:])
                for i in range(CT):
                    t = ch * CT + i
                    a1 = a1ld[:, i * TILE:(i + 1) * TILE]
                    nc.vector.tensor_scalar_add(a1, a1, delta[:, 0:1])
                    z2 = psB.tile([H, TILE], f32, space="PSUM", tag="z2")
                    for s in range(NS):
                        nc.tensor.matmul(
                            z2[:, s * H:(s + 1) * H],
                            lhsT=a1[:, s * SUB:(s + 1) * SUB],
                            rhs=w1p[:], start=True, stop=True)
                    # a2t: per subtile [bei (32) | ones (1) | a2 (128)]
                    a2t = a2pool.tile([128, NS * GW], dt_c, tag="a2t")
                    a2t3 = a2t[:].rearrange("p (g c) -> p g c", c=GW)
                    nc.vector.tensor_copy(
                        a2t3[:, :, 0:BW],
                        bei_sb[:, (t * NS) * BW:(t * NS + NS) * BW]
                        .rearrange("p (g c) -> p g c", c=BW))
                    nc.scalar.activation(
                        a2t3[:, :, BW:GW],
                        z2[:].rearrange("p (g c) -> p g c", c=H),
                        mybir.ActivationFunctionType.Relu)
                    # zero a2 for pad edges (bei cols are host-zeroed)
                    for s in range(NS):
                        pl = esh_real - (t * NS + s) * SUB
                        if pl <= 0:
                            nc.vector.memset(a2t3[:, s, BW:GW], 0.0)
                        elif pl < SUB:
                            nc.vector.tensor_scalar_mul(
                                a2t3[:, s, BW:GW], a2t3[:, s, BW:GW],
                                edge_mask[:, 0:1])
                    first = (t == 0)
                    last = (t == NT - 1)
                    for s in range(NS):
                        nc.tensor.matmul(
                            gacc[:],
                            lhsT=a2t[:, s * GW + BW:(s + 1) * GW],
                            rhs=a2t[:, s * GW:(s + 1) * GW],
                            start=(first and s == 0),
                            stop=(last and s == NS - 1),
                            skip_group_check=True)

            # ---- AllReduce #2: [G^T | sum2 | sumsq2] ----
            garr = mpool.tile([H, BW + 1], f32, tag="garr")
            nc.vector.tensor_copy(garr[:, 0:BW], gacc[:, 0:BW])
            scr = mpool.tile([128, 128], f32, tag="scr")
            nc.vector.tensor_mul(scr[:], gacc[:, BW:GW], id128[:])
            nc.vector.tensor_reduce(
                garr[:, BW:BW + 1], scr[:], mybir.AxisListType.X,
                mybir.AluOpType.add)

            cc2_in = dpool.tile([H, BW + 1], f32)
            cc2_out = dpool.tile([H, BW + 1], f32)
            nc.sync.dma_start(cc2_in[:], garr[:])
            nc.gpsimd.collective_compute(
                "AllReduce", mybir.AluOpType.add, replica_groups=rg,
                ins=[cc2_in.opt()], outs=[cc2_out.opt()])
            gall = mpool.tile([H, BW + 1], f32, tag="gall")
            nc.sync.dma_start(gall[:], cc2_out[:])

            # ---- epilogue ----
            mu2 = mpool.tile([H, 1], f32, tag="mu")
            nc.scalar.mul(mu2[:], gall[:, B:B + 1], 1.0 / E_total)
            ex2b = mpool.tile([H, 1], f32, tag="ex2")
            nc.scalar.mul(ex2b[:], gall[:, BW:BW + 1], 1.0 / E_total)
            var2 = mpool.tile([H, 1], f32, tag="var")
            nc.vector.tensor_mul(var2[:], mu2[:], mu2[:])
            nc.vector.tensor_sub(var2[:], ex2b[:], var2[:])
            sd2 = mpool.tile([H, 1], f32, tag="sd")
            nc.vector.tensor_scalar_add(sd2[:], var2[:], EPS)
            nc.scalar.sqrt(sd2[:], sd2[:])
            isd2 = mpool.tile([H, 1], f32, tag="isd")
            nc.vector.reciprocal(isd2[:], sd2[:])
            s2 = mpool.tile([H, 1], f32, tag="s1")
            nc.vector.tensor_mul(s2[:], g1c[:], isd2[:])
            t2 = mpool.tile([H, 1], f32, tag="t1")
            nc.vector.tensor_mul(t2[:], mu2[:], s2[:])
            nc.vector.tensor_sub(t2[:], bt1c[:], t2[:])

            w2p = mpool.tile([H, KDIM], f32, tag="w2p")
            nc.vector.tensor_scalar_mul(w2p[:], w2sb[:], s2[:, 0:1])
            pr2 = psS.tile([1, KDIM], f32, space="PSUM", tag="pss")
            nc.tensor.matmul(pr2[:], lhsT=t2[:], rhs=w2sb[:], start=True, stop=True)
            b2p_row = mpool.tile([1, KDIM], f32, tag="b2pr")
            nc.vector.tensor_add(b2p_row[:], pr2[:], b2r[:])

            out_ps = psS.tile([B, KDIM], f32, space="PSUM", tag="pss")
            nc.tensor.matmul(out_ps[:], lhsT=gall[:, 0:B], rhs=w2p[:],
                             start=True, stop=False)
            nc.tensor.matmul(out_ps[:], lhsT=rsr[:], rhs=b2p_row[:],
                             start=False, stop=True)
            outsb = mpool.tile([B, KDIM], f32, tag="outsb")
            nc.vector.tensor_scalar_mul(outsb[:], out_ps[:], invc[:, 0:1])
            nc.sync.dma_start(outd[:], outsb[:])

    # Legalize waits for walrus (TRN2: max 1 wait/instruction; extras are
    # spilled onto ldweights / event-semaphore instructions).
    import bass_rust as _br
    _br.move_matmul_waits_to_ldweights(nc.m)
    _br.generate_event_semaphores(nc)
    nc.finalize()
    return nc


def _ceil_to(x, m):
    return (x + m - 1) // m * m


def make_inputs(inputs, ESH, N, dt_c=bf16, dt_en=None):
    """Host-side shard/layout prep. Returns in_maps for run_bass_kernel_spmd."""
    del N, dt_en
    np_c = _np_dt(dt_c)
    en = np.asarray(inputs["edge_nodes"], dtype=np.float32)
    x = np.asarray(inputs["edge_feats"], dtype=np.float32)
    src = np.asarray(inputs["src"]).astype(np.int64)
    dst = np.asarray(inputs["dst"]).astype(np.int64)
    E = x.shape[0]
    Nn = en.shape[1]
    esh_real = E // NCORES
    NSUB = ESH // SUB

    # exact row sums via degree counts (en entries are 0/1)
    deg = (np.bincount(src, minlength=Nn) + np.bincount(dst, minlength=Nn))
    rs = en.astype(np.float64) @ deg.astype(np.float64)
    inv = np.where(rs > 0, 1.0 / np.where(rs > 0, rs, 1.0), 0.0)

    enT = en.T  # [N, B]

    common = dict(
        W0=np.vstack([np.asarray(inputs["W0"], np.float32)] * 2).astype(np_c),
        W1=np.asarray(inputs["W1"], np.float32),
        W2=np.asarray(inputs["W2"], np.float32),
        b0c=np.asarray(inputs["b0"], np.float32).reshape(H, 1),
        b2r=np.asarray(inputs["b2"], np.float32).reshape(1, KDIM),
        g0c=np.asarray(inputs["g0"], np.float32).reshape(H, 1),
        bt0c=np.asarray(inputs["bt0"], np.float32).reshape(H, 1),
        g1c=np.asarray(inputs["g1"], np.float32).reshape(H, 1),
        bt1c=np.asarray(inputs["bt1"], np.float32).reshape(H, 1),
        rsr=rs.astype(np.float32).reshape(1, B),
        invc=inv.astype(np.float32).reshape(B, 1),
        qc=np.linalg.solve(
            np.asarray(inputs["W1"], np.float64).T,
            np.asarray(inputs["b1"], np.float64),
        ).astype(np.float32).reshape(H, 1),
    )

    in_maps = []
    for c in range(NCORES):
        lo = c * esh_real
        xs = x[lo:lo + esh_real]
        xT = np.zeros((D, ESH), np.float32)
        xT[:, :esh_real] = xs.T
        NTP = ESH // (2 * TILE)
        xTi = np.ascontiguousarray(
            xT.reshape(D, NTP, 2, TILE).transpose(2, 0, 1, 3).reshape(128, ESH // 2)
        ).astype(np_c)

        bei_aug = np.zeros((ESH, BW), np.float32)
        bei_aug[:esh_real, 0:B] = enT[src[lo:lo + esh_real]] + enT[dst[lo:lo + esh_real]]
        bei_aug[:esh_real, B] = 1.0
        beiT = np.ascontiguousarray(
            bei_aug.reshape(NSUB, 128, BW).transpose(1, 0, 2).reshape(128, NSUB * BW)
        ).astype(np_c)

        in_maps.append(dict(common, xTi=xTi, beiT=beiT))
    return in_maps


_NC_CACHE = {}


def kernel(**inputs):
    dt_c = bf16 if COMPUTE_DT == "bf16" else f32
    x = np.asarray(inputs["edge_feats"])
    en = np.asarray(inputs["edge_nodes"])
    E = x.shape[0]
    N = en.shape[1]
    ESH = _ceil_to(E // NCORES, GATHER_BATCH)
    key = (ESH, N, E, COMPUTE_DT)
    if key not in _NC_CACHE:
        _NC_CACHE[key] = build_nc(ESH, N, E, dt_c=dt_c)
    nc = _NC_CACHE[key]
    in_maps = make_inputs(inputs, ESH, N, dt_c=dt_c)
    res = run_bass_kernel_spmd(nc, in_maps, list(range(NCORES)))
    return np.asarray(res.results[0]["out"], np.float32)


# revision 38
# speedup vs baseline: 1.1102x; 1.0093x over previous
"""Trainium2 Bass kernel for DeepEdgeConvolution (gnn_message_passing).

Math (reference):
    bei = edge_nodes[:, src] + edge_nodes[:, dst]          # [B, E]
    bei = bei / row_sum (0 if empty row)
    h = BN1(relu(x @ W0 + b0)); h = BN2(relu(h @ W1 + b1)); h = h @ W2 + b2
    out = bei @ h                                          # [B, K]

Restructured: fold BN1 into (W1, b1) and BN2 into (W2, b2):
    a1 = relu(x @ W0 + b0)             (BN1 stats over E -> s1, t1)
    W1' = diag(s1) W1 ; b1' = t1 @ W1 + b1
    a2 = relu(a1 @ W1' + b1')          (BN2 stats over E -> s2, t2)
    W2' = diag(s2) W2 ; b2' = t2 @ W2 + b2
    out = diag(inv) [ (bei_raw @ a2) @ W2' + rs_raw x b2' ]

Sharding: edges across 8 cores; two streaming passes over x^T per core
(pass A: BN1 stats via bn_stats; pass B: recompute a1, then a2, G).

bei is computed on the HOST (sharding the columns of batch_edge_idcs per the
sharding hint): beiT is streamed as a dense packed input [128, NSUB*33] where
each 33-col block is [bei | 1] for one 128-edge subtile (edges on partitions).
Pad edges get all-zero columns (including the ones entry), which kills every
pad correction. Row sums rs / inv are exact small host-side inputs.

G accumulation (the [B,E]x[E,K] spmm): per 128-edge subtile,
    psG += a2_sub^T @ [bei_aug | a2_sub]    -> [H, 33 + H]
giving G^T (cols 0:32), sum(a2) (col 32, via the ones column) and the a2 Gram
matrix whose diagonal is sum(a2^2) -- one PSUM chain yields everything BN2 and
the final matmul need.  One AllReduce of [H, 34] follows; the epilogue is a
couple of tiny matmuls.
"""

import numpy as np

import concourse.bacc as bacc
import concourse.bass as bass
import concourse.tile as tile
from concourse import mybir
from concourse.bass_utils import run_bass_kernel_spmd
from concourse.masks import make_identity

f32 = mybir.dt.float32
bf16 = mybir.dt.bfloat16
i32 = mybir.dt.int32

NCORES = 8
B, D, H, KDIM = 32, 64, 128, 128
EPS = 1e-5
TILE = 512           # edges per tile
SUB = 128            # edges per matmul subtile
GATHER_BATCH = 2048  # edges per DMA chunk (ESH must be a multiple)

# compute dtype: "f32" (exact-ish) or "bf16" (fast).
COMPUTE_DT = "bf16"

BW = B + 1           # bei block width: [bei (32) | ones (1)]


def _np_dt(dt):
    if dt == bf16:
        import ml_dtypes
        return ml_dtypes.bfloat16
    return np.float32


def build_nc(ESH, N, E_total, dt_c=bf16, dt_en=None, debug=False):
    """Build the SPMD Bass program. ESH = padded edges per core."""
    del N, dt_en, debug
    assert ESH % GATHER_BATCH == 0
    NT = ESH // TILE          # tiles per core
    NSUB = ESH // SUB         # 128-edge subtiles per core
    NCH = ESH // GATHER_BATCH  # DMA chunks per core
    esh_real = E_total // NCORES
    assert E_total % NCORES == 0
    NS = TILE // SUB          # subtiles per tile (4)
    GW = BW + H               # gacc rhs width: [bei | 1 | a2]
    CCOL = GATHER_BATCH // 2  # packed x cols per chunk
    CT = GATHER_BATCH // TILE  # tiles per chunk (4)
    # last PCH chunks: their x stays resident in SBUF; z1/a1 is recomputed
    # from it between pass A and the BN1 fold (AllReduce-independent PE work
    # that hides the collective latency with no competing DMA traffic); these
    # chunks skip the DRAM a1 spill.
    PCH = min(12, NCH - 1) if NCH > 1 else 0

    nc = bass.Bass()

    # ---- I/O ----
    xTi = nc.dram_tensor("xTi", [128, ESH // 2], dt_c, kind="ExternalInput")
    beiT = nc.dram_tensor("beiT", [128, NSUB * BW], dt_c, kind="ExternalInput")
    W0d = nc.dram_tensor("W0", [2 * D, H], dt_c, kind="ExternalInput")
    W1d = nc.dram_tensor("W1", [H, H], f32, kind="ExternalInput")
    W2d = nc.dram_tensor("W2", [H, KDIM], f32, kind="ExternalInput")
    b0cd = nc.dram_tensor("b0c", [H, 1], f32, kind="ExternalInput")
    b2rd = nc.dram_tensor("b2r", [1, KDIM], f32, kind="ExternalInput")
    g0cd = nc.dram_tensor("g0c", [H, 1], f32, kind="ExternalInput")
    bt0cd = nc.dram_tensor("bt0c", [H, 1], f32, kind="ExternalInput")
    g1cd = nc.dram_tensor("g1c", [H, 1], f32, kind="ExternalInput")
    bt1cd = nc.dram_tensor("bt1c", [H, 1], f32, kind="ExternalInput")
    rsrd = nc.dram_tensor("rsr", [1, B], f32, kind="ExternalInput")
    invcd = nc.dram_tensor("invc", [B, 1], f32, kind="ExternalInput")
    qcd = nc.dram_tensor("qc", [H, 1], f32, kind="ExternalInput")
    outd = nc.dram_tensor("out", [B, KDIM], f32, kind="ExternalOutput")

    rg = [list(range(NCORES))]

    with tile.TileContext(nc) as tc:
        with (
            tc.tile_pool(name="const", bufs=1) as cpool,
            tc.tile_pool(name="xp", bufs=14) as xpool,
            tc.tile_pool(name="a1s", bufs=3) as a1spool,
            tc.tile_pool(name="a1l", bufs=4) as a1lpool,
            tc.tile_pool(name="a1k", bufs=12) as a1kpool,
            tc.tile_pool(name="a2p", bufs=3) as a2pool,
            tc.tile_pool(name="misc", bufs=2) as mpool,
            tc.tile_pool(name="psA", bufs=4, space="PSUM") as psA,
            tc.tile_pool(name="psB", bufs=2, space="PSUM") as psB,
            tc.tile_pool(name="psG", bufs=1, space="PSUM") as psG,
            tc.tile_pool(name="psS", bufs=1, space="PSUM") as psS,
            tc.tile_pool(name="dram", bufs=1, space="DRAM") as dpool,
        ):
            # ---- constants / params in SBUF ----
            w0sb = cpool.tile([128, H], dt_c)  # W0 duplicated on both halves
            nc.sync.dma_start(w0sb[:], W0d[:])
            b0c = cpool.tile([H, 1], f32)
            nc.sync.dma_start(b0c[:], b0cd[:])

            stats1 = cpool.tile([H, 6 * NT], f32)

            # bei resident in SBUF (DMA issues interleaved into pass A so the
            # SP issue queue doesn't delay the first x chunk)
            bei_sb = cpool.tile([128, NSUB * BW], dt_c)
            BCW = (GATHER_BATCH // SUB) * BW

            # a1 spill buffers in DRAM, one per chunk (precise DMA deps)
            a1d = [dpool.tile([128, GATHER_BATCH], dt_c, name=f"a1d{ch}")
                   for ch in range(NCH)]

            # ================= PASS A: BN1 stats; a1 spilled to DRAM ========
            xs_keep = {}
            for ch in range(NCH):
                xch = xpool.tile([128, CCOL], dt_c, tag="xch")
                nc.sync.dma_start(xch[:], xTi[:, ch * CCOL:(ch + 1) * CCOL])
                nc.sync.dma_start(
                    bei_sb[:, ch * BCW:(ch + 1) * BCW],
                    beiT[:, ch * BCW:(ch + 1) * BCW])
                a1ch = a1spool.tile([H, GATHER_BATCH], dt_c, tag="a1ch")
                for tp in range(GATHER_BATCH // (2 * TILE)):
                    for u in range(2):
                        t = ch * CT + 2 * tp + u
                        i = 2 * tp + u
                        z1 = psA.tile([H, TILE], f32, space="PSUM", tag="z1")
                        nc.tensor.matmul(
                            z1[:], lhsT=w0sb[u * D:(u + 1) * D, :],
                            rhs=xch[u * D:(u + 1) * D, tp * TILE:(tp + 1) * TILE],
                            start=True, stop=True)
                        a1 = a1ch[:, i * TILE:(i + 1) * TILE]
                        nc.scalar.activation(
                            a1, z1[:], mybir.ActivationFunctionType.Relu,
                            bias=b0c[:, 0:1])
                        pad_lo = esh_real - t * TILE
                        if pad_lo < TILE:
                            nc.vector.memset(
                                a1ch[:, i * TILE + max(pad_lo, 0):
                                     (i + 1) * TILE], 0.0)
                        nc.vector.bn_stats(stats1[:, 6 * t:6 * t + 6], a1)
                if ch >= NCH - PCH:
                    xs_keep[ch] = xch
                else:
                    nc.sync.dma_start(a1d[ch][:], a1ch[:])

            # ---- AllReduce #1: BN1 sums (issued before the prelude so the
            # collective overlaps with pass-B z1/a1 work) ----
            mv1 = mpool.tile([H, 2], f32, tag="mv")
            nc.vector.bn_aggr(mv1[:], stats1[:])
            # raw sums over this shard (pads are zero -> exact)
            ar1 = mpool.tile([H, 2], f32, tag="ar")
            nc.scalar.mul(ar1[:, 0:1], mv1[:, 0:1], float(ESH))
            msq1 = mpool.tile([H, 1], f32, tag="msq")
            nc.vector.tensor_mul(msq1[:], mv1[:, 0:1], mv1[:, 0:1])
            nc.vector.tensor_add(msq1[:], msq1[:], mv1[:, 1:2])
            nc.scalar.mul(ar1[:, 1:2], msq1[:], float(ESH))

            cc1_in = dpool.tile([H, 2], f32)
            cc1_out = dpool.tile([H, 2], f32)
            nc.sync.dma_start(cc1_in[:], ar1[:])
            nc.gpsimd.collective_compute(
                "AllReduce", mybir.AluOpType.add, replica_groups=rg,
                ins=[cc1_in.opt()], outs=[cc1_out.opt()])
            gs1 = mpool.tile([H, 2], f32, tag="gs")
            nc.sync.dma_start(gs1[:], cc1_out[:])

            # ---- prelude: recompute z1/a1 of the kept chunks while the
            # collective is in flight (x already resident, no DMA) ----
            a1_keep = {}
            for ch in sorted(xs_keep):
                xch = xs_keep.pop(ch)
                a1k = a1kpool.tile([H, GATHER_BATCH], dt_c, tag="a1k")
                for tp in range(GATHER_BATCH // (2 * TILE)):
                    for u in range(2):
                        i = 2 * tp + u
                        z1 = psA.tile([H, TILE], f32, space="PSUM", tag="z1")
                        nc.tensor.matmul(
                            z1[:], lhsT=w0sb[u * D:(u + 1) * D, :],
                            rhs=xch[u * D:(u + 1) * D, tp * TILE:(tp + 1) * TILE],
                            start=True, stop=True)
                        nc.vector.tensor_scalar(
                            a1k[:, i * TILE:(i + 1) * TILE], z1[:],
                            b0c[:, 0:1], 0.0,
                            op0=mybir.AluOpType.add, op1=mybir.AluOpType.max)
                a1_keep[ch] = a1k

            # ---- deferred params (not needed until the fold / epilogue) ----
            w1sb = cpool.tile([H, H], f32)
            nc.sync.dma_start(w1sb[:], W1d[:])
            w2sb = cpool.tile([H, KDIM], f32)
            nc.sync.dma_start(w2sb[:], W2d[:])
            b2r = cpool.tile([1, KDIM], f32)
            nc.sync.dma_start(b2r[:], b2rd[:])
            g0c = cpool.tile([H, 1], f32)
            nc.sync.dma_start(g0c[:], g0cd[:])
            bt0c = cpool.tile([H, 1], f32)
            nc.sync.dma_start(bt0c[:], bt0cd[:])
            g1c = cpool.tile([H, 1], f32)
            nc.sync.dma_start(g1c[:], g1cd[:])
            bt1c = cpool.tile([H, 1], f32)
            nc.sync.dma_start(bt1c[:], bt1cd[:])
            rsr = cpool.tile([1, B], f32)
            nc.sync.dma_start(rsr[:], rsrd[:])
            invc = cpool.tile([B, 1], f32)
            nc.sync.dma_start(invc[:], invcd[:])
            qc = cpool.tile([H, 1], f32)
            nc.sync.dma_start(qc[:], qcd[:])
            id128 = cpool.tile([128, 128], f32)
            make_identity(nc, id128[:])
            pad_frac = esh_real % SUB
            edge_mask = None
            if pad_frac:
                pidx = cpool.tile([128, 1], i32)
                nc.gpsimd.iota(pidx[:], pattern=[[0, 1]], base=0,
                               channel_multiplier=1)
                pidx_f = cpool.tile([128, 1], f32)
                nc.vector.tensor_copy(pidx_f[:], pidx[:])
                edge_mask = cpool.tile([128, 1], f32)
                nc.vector.tensor_scalar(
                    edge_mask[:], pidx_f[:], float(pad_frac), None,
                    op0=mybir.AluOpType.is_lt)

            # mu, var, s1, t1
            mu1 = mpool.tile([H, 1], f32, tag="mu")
            nc.scalar.mul(mu1[:], gs1[:, 0:1], 1.0 / E_total)
            ex2 = mpool.tile([H, 1], f32, tag="ex2")
            nc.scalar.mul(ex2[:], gs1[:, 1:2], 1.0 / E_total)
            var1 = mpool.tile([H, 1], f32, tag="var")
            nc.vector.tensor_mul(var1[:], mu1[:], mu1[:])
            nc.vector.tensor_sub(var1[:], ex2[:], var1[:])
            sd1 = mpool.tile([H, 1], f32, tag="sd")
            nc.vector.tensor_scalar_add(sd1[:], var1[:], EPS)
            nc.scalar.sqrt(sd1[:], sd1[:])
            isd1 = mpool.tile([H, 1], f32, tag="isd")
            nc.vector.reciprocal(isd1[:], sd1[:])
            s1 = mpool.tile([H, 1], f32, tag="s1")
            nc.vector.tensor_mul(s1[:], g0c[:], isd1[:])
            t1 = mpool.tile([H, 1], f32, tag="t1")
            nc.vector.tensor_mul(t1[:], mu1[:], s1[:])
            nc.vector.tensor_sub(t1[:], bt0c[:], t1[:])

            # W1' (compute dtype); bias b1' enters z2 through the a1 shift
            # delta = diag(1/s1)(t1 + W1^-T b1), since
            # (relu(z1+b0) + delta)^T W1' = a1^T W1' + b1'  and
            # relu(z1+b0) + delta = max(z1 + (b0+delta), delta).
            w1p = cpool.tile([H, H], dt_c)
            nc.vector.tensor_scalar_mul(w1p[:], w1sb[:], s1[:, 0:1])
            is1 = mpool.tile([H, 1], f32, tag="is1")
            nc.vector.reciprocal(is1[:], s1[:])
            delta = cpool.tile([H, 1], f32)
            nc.vector.tensor_add(delta[:], t1[:], qc[:])
            nc.vector.tensor_mul(delta[:], delta[:], is1[:])

            # ============ PASS B: a2, G^T / sum2 / Gram accumulation ============
            gacc = psG.tile([H, GW], f32, space="PSUM", tag="gacc")
            for ch in range(NCH):
                if ch in a1_keep:
                    a1ld = a1_keep.pop(ch)
                else:
                    a1ld = a1lpool.tile([H, GATHER_BATCH], dt_c, tag="a1ld")
                    nc.sync.dma_start(a1ld[:], a1d[ch][:])
                for i in range(CT):
                    t = ch * CT + i
                    a1 = a1ld[:, i * TILE:(i + 1) * TILE]
                    nc.vector.tensor_scalar_add(a1, a1, delta[:, 0:1])
                    z2 = psB.tile([H, TILE], f32, space="PSUM", tag="z2")
                    for s in range(NS):
                        nc.tensor.matmul(
                            z2[:, s * H:(s + 1) * H],
                            lhsT=a1[:, s * SUB:(s + 1) * SUB],
                            rhs=w1p[:], start=True, stop=True)
                    # a2t: per subtile [bei (32) | ones (1) | a2 (128)]
                    a2t = a2pool.tile([128, NS * GW], dt_c, tag="a2t")
                    a2t3 = a2t[:].rearrange("p (g c) -> p g c", c=GW)
                    nc.vector.tensor_copy(
                        a2t3[:, :, 0:BW],
                        bei_sb[:, (t * NS) * BW:(t * NS + NS) * BW]
                        .rearrange("p (g c) -> p g c", c=BW))
                    nc.scalar.activation(
                        a2t3[:, :, BW:GW],
                        z2[:].rearrange("p (g c) -> p g c", c=H),
                        mybir.ActivationFunctionType.Relu)
                    # zero a2 for pad edges (bei cols are host-zeroed)
                    for s in range(NS):
                        pl = esh_real - (t * NS + s) * SUB
                        if pl <= 0:
                            nc.vector.memset(a2t3[:, s, BW:GW], 0.0)
                        elif pl < SUB:
                            nc.vector.tensor_scalar_mul(
                                a2t3[:, s, BW:GW], a2t3[:, s, BW:GW],
                                edge_mask[:, 0:1])
                    first = (t == 0)
                    last = (t == NT - 1)
                    for s in range(NS):
                        nc.tensor.matmul(
                            gacc[:],
                            lhsT=a2t[:, s * GW + BW:(s + 1) * GW],
                            rhs=a2t[:, s * GW:(s + 1) * GW],
                            start=(first and s == 0),
                            stop=(last and s == NS - 1),
                            skip_group_check=True)

            # ---- AllReduce #2: [G^T | sum2 | sumsq2] ----
            garr = mpool.tile([H, BW + 1], f32, tag="garr")
            nc.vector.tensor_copy(garr[:, 0:BW], gacc[:, 0:BW])
            scr = mpool.tile([128, 128], f32, tag="scr")
            nc.vector.tensor_mul(scr[:], gacc[:, BW:GW], id128[:])
            nc.vector.tensor_reduce(
                garr[:, BW:BW + 1], scr[:], mybir.AxisListType.X,
                mybir.AluOpType.add)

            cc2_in = dpool.tile([H, BW + 1], f32)
            cc2_out = dpool.tile([H, BW + 1], f32)
            nc.sync.dma_start(cc2_in[:], garr[:])
            nc.gpsimd.collective_compute(
                "AllReduce", mybir.AluOpType.add, replica_groups=rg,
                ins=[cc2_in.opt()], outs=[cc2_out.opt()])
            gall = mpool.tile([H, BW + 1], f32, tag="gall")
            nc.sync.dma_start(gall[:], cc2_out[:])

            # ---- epilogue ----
            mu2 = mpool.tile([H, 1], f32, tag="mu")
            nc.scalar.mul(mu2[:], gall[:, B:B + 1], 1.0 / E_total)
            ex2b = mpool.tile([H, 1], f32, tag="ex2")
            nc.scalar.mul(ex2b[:], gall[:, BW:BW + 1], 1.0 / E_total)
            var2 = mpool.tile([H, 1], f32, tag="var")
            nc.vector.tensor_mul(var2[:], mu2[:], mu2[:])
            nc.vector.tensor_sub(var2[:], ex2b[:], var2[:])
            sd2 = mpool.tile([H, 1], f32, tag="sd")
            nc.vector.tensor_scalar_add(sd2[:], var2[:], EPS)
            nc.scalar.sqrt(sd2[:], sd2[:])
            isd2 = mpool.tile([H, 1], f32, tag="isd")
            nc.vector.reciprocal(isd2[:], sd2[:])
            s2 = mpool.tile([H, 1], f32, tag="s1")
            nc.vector.tensor_mul(s2[:], g1c[:], isd2[:])
            t2 = mpool.tile([H, 1], f32, tag="t1")
            nc.vector.tensor_mul(t2[:], mu2[:], s2[:])
            nc.vector.tensor_sub(t2[:], bt1c[:], t2[:])

            w2p = mpool.tile([H, KDIM], f32, tag="w2p")
            nc.vector.tensor_scalar_mul(w2p[:], w2sb[:], s2[:, 0:1])
            pr2 = psS.tile([1, KDIM], f32, space="PSUM", tag="pss")
            nc.tensor.matmul(pr2[:], lhsT=t2[:], rhs=w2sb[:], start=True, stop=True)
            b2p_row = mpool.tile([1, KDIM], f32, tag="b2pr")
            nc.vector.tensor_add(b2p_row[:], pr2[:], b2r[:])

            out_ps = psS.tile([B, KDIM], f32, space="PSUM", tag="pss")
            nc.tensor.matmul(out_ps[:], lhsT=gall[:, 0:B], rhs=w2p[:],
                             start=True, stop=False)
            nc.tensor.matmul(out_ps[:], lhsT=rsr[:], rhs=b2p_row[:],
                             start=False, stop=True)
            outsb = mpool.tile([B, KDIM], f32, tag="outsb")
            nc.vector.tensor_scalar_mul(outsb[:], out_ps[:], invc[:, 0:1])
            nc.sync.dma_start(outd[:], outsb[:])

    # Legalize waits for walrus (TRN2: max 1 wait/instruction; extras are
    # spilled onto ldweights / event-semaphore instructions).
    import bass_rust as _br
    _br.move_matmul_waits_to_ldweights(nc.m)
    _br.generate_event_semaphores(nc)
    nc.finalize()
    return nc


def _ceil_to(x, m):
    return (x + m - 1) // m * m


def make_inputs(inputs, ESH, N, dt_c=bf16, dt_en=None):
    """Host-side shard/layout prep. Returns in_maps for run_bass_kernel_spmd."""
    del N, dt_en
    np_c = _np_dt(dt_c)
    en = np.asarray(inputs["edge_nodes"], dtype=np.float32)
    x = np.asarray(inputs["edge_feats"], dtype=np.float32)
    src = np.asarray(inputs["src"]).astype(np.int64)
    dst = np.asarray(inputs["dst"]).astype(np.int64)
    E = x.shape[0]
    Nn = en.shape[1]
    esh_real = E // NCORES
    NSUB = ESH // SUB

    # exact row sums via degree counts (en entries are 0/1)
    deg = (np.bincount(src, minlength=Nn) + np.bincount(dst, minlength=Nn))
    rs = en.astype(np.float64) @ deg.astype(np.float64)
    inv = np.where(rs > 0, 1.0 / np.where(rs > 0, rs, 1.0), 0.0)

    enT = en.T  # [N, B]

    common = dict(
        W0=np.vstack([np.asarray(inputs["W0"], np.float32)] * 2).astype(np_c),
        W1=np.asarray(inputs["W1"], np.float32),
        W2=np.asarray(inputs["W2"], np.float32),
        b0c=np.asarray(inputs["b0"], np.float32).reshape(H, 1),
        b2r=np.asarray(inputs["b2"], np.float32).reshape(1, KDIM),
        g0c=np.asarray(inputs["g0"], np.float32).reshape(H, 1),
        bt0c=np.asarray(inputs["bt0"], np.float32).reshape(H, 1),
        g1c=np.asarray(inputs["g1"], np.float32).reshape(H, 1),
        bt1c=np.asarray(inputs["bt1"], np.float32).reshape(H, 1),
        rsr=rs.astype(np.float32).reshape(1, B),
        invc=inv.astype(np.float32).reshape(B, 1),
        qc=np.linalg.solve(
            np.asarray(inputs["W1"], np.float64).T,
            np.asarray(inputs["b1"], np.float64),
        ).astype(np.float32).reshape(H, 1),
    )

    in_maps = []
    for c in range(NCORES):
        lo = c * esh_real
        xs = x[lo:lo + esh_real]
        xT = np.zeros((D, ESH), np.float32)
        xT[:, :esh_real] = xs.T
        NTP = ESH // (2 * TILE)
        xTi = np.ascontiguousarray(
            xT.reshape(D, NTP, 2, TILE).transpose(2, 0, 1, 3).reshape(128, ESH // 2)
        ).astype(np_c)

        bei_aug = np.zeros((ESH, BW), np.float32)
        bei_aug[:esh_real, 0:B] = enT[src[lo:lo + esh_real]] + enT[dst[lo:lo + esh_real]]
        bei_aug[:esh_real, B] = 1.0
        beiT = np.ascontiguousarray(
            bei_aug.reshape(NSUB, 128, BW).transpose(1, 0, 2).reshape(128, NSUB * BW)
        ).astype(np_c)

        in_maps.append(dict(common, xTi=xTi, beiT=beiT))
    return in_maps


_NC_CACHE = {}


def kernel(**inputs):
    dt_c = bf16 if COMPUTE_DT == "bf16" else f32
    x = np.asarray(inputs["edge_feats"])
    en = np.asarray(inputs["edge_nodes"])
    E = x.shape[0]
    N = en.shape[1]
    ESH = _ceil_to(E // NCORES, GATHER_BATCH)
    key = (ESH, N, E, COMPUTE_DT)
    if key not in _NC_CACHE:
        _NC_CACHE[key] = build_nc(ESH, N, E, dt_c=dt_c)
    nc = _NC_CACHE[key]
    in_maps = make_inputs(inputs, ESH, N, dt_c=dt_c)
    res = run_bass_kernel_spmd(nc, in_maps, list(range(NCORES)))
    return np.asarray(res.results[0]["out"], np.float32)
